# revision 1
# baseline (speedup 1.0000x reference)
# Bass/Tile kernel for nn_EquiConv (gnn_message_passing, memory-bound).
#
# Math (per edge e), with w2_* path scales and e3nn norms folded into weights:
#   s1 = x1[:, :128], v1[u,m] = x1[:, 128+3u+m], s2 = x2[:,0], v2m = x2[:,1+m]
#   out0 = (s1*s2) @ W1 + sum_m (v1m*v2m) @ W4        [E,128]
#   out1m = (s1*v2m) @ W2 + (v1m*s2) @ W3             [E,64] for m=0,1,2
#   w = F2 @ silu(F1 @ silu(F0 @ fw))                 [E,192]
#   res[:, :128] = out0 * w[:, :128]
#   res[:, 128+3w+m] = out1m[:, w] * w[:, 128+w]
#
# Strategy: edge-data-parallel across 8 cores. Per core, tiles of 256 edges
# (2 blocks of 128). Edge-major prescale (tensor_scalar with per-partition
# scalar = per-edge), PE transposes to feature-major, PSUM-accumulated bf16
# matmuls with stationary weights, per-edge FC weights via 3-layer MLP,
# final elementwise on DVE, PE transpose back to edge-major with strided
# PSUM writes producing the interleaved 1o layout directly.

import numpy as np
import ml_dtypes
from contextlib import ExitStack

import concourse.bass as bass
import concourse.tile as tile
from concourse import bacc, mybir
from concourse.bass_utils import run_bass_kernel_spmd

E_TOTAL = 262144
N_CORES = 8
E_CORE = E_TOTAL // N_CORES   # 32768
TILE_E = 256                  # edges per tile (2 blocks of 128)
M0, M1 = 128, 64
BF16 = mybir.dt.bfloat16
F32 = mybir.dt.float32
# module-level so the sim test can swap in a CoreSim-implemented function
ACT_FN = mybir.ActivationFunctionType.Silu
# timing-variant gate: 1=DMA only, 2=+prescale, 3=+transpose/evac,
# 4=+matmuls+res, 5=full pipeline (default), 6=full minus FC (dummy w)
VAR = 5
# psum pool bufs: (t1, t2, mm1, mm2, mm3, ob) — must total <= 8 banks
PSUM_BUFS = (2, 2, 1, 1, 1, 1)
SPLIT_EVAC = False
GRP_N = 4

INV_SQRT3 = 1.0 / np.sqrt(3.0)
C0 = np.sqrt(1.0 / 192.0)
C1 = np.sqrt(3.0 / 192.0)


def build_nc(e_core=E_CORE, num_devices=N_CORES, repeat=1):
    nc = bacc.Bacc("TRN2", target_bir_lowering=False, debug=False,
                   num_devices=num_devices)
    x1 = nc.dram_tensor("x1", [e_core, 320], F32, kind="ExternalInput").ap()
    x2 = nc.dram_tensor("x2", [e_core, 4], F32, kind="ExternalInput").ap()
    fw = nc.dram_tensor("fw", [e_core, 128], F32, kind="ExternalInput").ap()
    wW1 = nc.dram_tensor("wW1", [128, 128], BF16, kind="ExternalInput").ap()
    wW2 = nc.dram_tensor("wW2", [128, 64], BF16, kind="ExternalInput").ap()
    wW3 = nc.dram_tensor("wW3", [64, 64], BF16, kind="ExternalInput").ap()
    wW4 = nc.dram_tensor("wW4", [64, 128], BF16, kind="ExternalInput").ap()
    wF0 = nc.dram_tensor("wF0", [128, 64], BF16, kind="ExternalInput").ap()
    wF1 = nc.dram_tensor("wF1", [64, 64], BF16, kind="ExternalInput").ap()
    wF2 = nc.dram_tensor("wF2", [64, 192], BF16, kind="ExternalInput").ap()
    ident = nc.dram_tensor("ident", [128, 128], BF16, kind="ExternalInput").ap()
    out = nc.dram_tensor("out", [e_core, 320], F32, kind="ExternalOutput").ap()

    with tile.TileContext(nc) as tc, ExitStack() as ctx:
        _body(ctx, tc, x1, x2, fw,
              dict(wW1=wW1, wW2=wW2, wW3=wW3, wW4=wW4,
                   wF0=wF0, wF1=wF1, wF2=wF2, ident=ident),
              out, e_core, repeat)
    nc.compile()
    return nc


def _body(ctx, tc, x1, x2, fw, w_aps, out, e_core, repeat=1):
    nc = tc.nc
    n_tiles = e_core // TILE_E

    const = ctx.enter_context(tc.tile_pool(name="const", bufs=1))
    cW1 = const.tile([128, 128], BF16)
    cW2 = const.tile([128, 64], BF16)
    c34 = const.tile([128, 128], BF16)   # W3 at [0:64,0:64], W4 at [64:128,0:128]
    cF0 = const.tile([128, 64], BF16)
    cF1 = const.tile([128, 64], BF16)    # F1 stored at partitions [64:128]
    cF2 = const.tile([64, 192], BF16)    # F2a = [:, 0:128], F2b = [:, 128:192]
    cId = const.tile([128, 128], BF16)

    nc.sync.dma_start(out=cW1[:], in_=w_aps["wW1"])
    nc.sync.dma_start(out=cW2[:], in_=w_aps["wW2"])
    nc.sync.dma_start(out=c34[0:64, 0:64], in_=w_aps["wW3"])
    nc.sync.dma_start(out=c34[64:128, 0:128], in_=w_aps["wW4"])
    nc.sync.dma_start(out=cF0[:], in_=w_aps["wF0"])
    nc.sync.dma_start(out=cF1[64:128, :], in_=w_aps["wF1"])
    nc.sync.dma_start(out=cF2[:], in_=w_aps["wF2"])
    nc.sync.dma_start(out=cId[:], in_=w_aps["ident"])

    inp = ctx.enter_context(tc.tile_pool(name="inp", bufs=4))
    pre = ctx.enter_context(tc.tile_pool(name="pre", bufs=3))
    evac = ctx.enter_context(tc.tile_pool(name="evac", bufs=3))
    fcs = ctx.enter_context(tc.tile_pool(name="fcs", bufs=3))
    ress = ctx.enter_context(tc.tile_pool(name="ress", bufs=3))
    obs = ctx.enter_context(tc.tile_pool(name="obs", bufs=3))

    bt1, bt2, bm1, bm2, bm3, bob = PSUM_BUFS
    pt1 = ctx.enter_context(tc.tile_pool(name="pt1", bufs=bt1, space="PSUM"))
    pt2 = ctx.enter_context(tc.tile_pool(name="pt2", bufs=bt2, space="PSUM"))
    pm1 = ctx.enter_context(tc.tile_pool(name="pm1", bufs=bm1, space="PSUM"))
    pm2 = ctx.enter_context(tc.tile_pool(name="pm2", bufs=bm2, space="PSUM"))
    pm3 = ctx.enter_context(tc.tile_pool(name="pm3", bufs=bm3, space="PSUM"))
    pob = ctx.enter_context(tc.tile_pool(name="pob", bufs=bob, space="PSUM"))

    # repeat>1 wraps the whole body in a HW loop — used only for timing runs
    # (device wall-clock isolation); the graded path uses repeat=1 (no loop).
    import contextlib
    GRP = min(GRP_N, n_tiles)  # tiles per DMA group (batched DMA amortizes SWDGE)
    assert n_tiles % GRP == 0
    loop_cm = tc.For_i(0, repeat, 1) if repeat > 1 else contextlib.nullcontext()
    with loop_cm:
     for g in range(n_tiles // GRP):
      ge0 = g * GRP * TILE_E
      x1s = inp.tile([128, 2 * GRP, 320], BF16)
      nc.gpsimd.dma_start(
          out=x1s[:],
          in_=x1[ge0:ge0 + GRP * TILE_E, :].rearrange("(n p) d -> p n d", p=128))
      x2s = inp.tile([128, 2 * GRP, 4], F32)
      nc.sync.dma_start(
          out=x2s[:],
          in_=x2[ge0:ge0 + GRP * TILE_E, :].rearrange("(n p) d -> p n d", p=128))
      fws = inp.tile([128, 2 * GRP, 128], BF16)
      nc.gpsimd.dma_start(
          out=fws[:],
          in_=fw[ge0:ge0 + GRP * TILE_E, :].rearrange("(n p) d -> p n d", p=128))
      obsg = obs.tile([128, 2 * GRP, 320], BF16)

      if VAR == 1:
          nc.gpsimd.dma_start(
              out=out[ge0:ge0 + GRP * TILE_E, :].rearrange("(n p) d -> p n d", p=128),
              in_=x1s[:])
          continue

      for tg in range(GRP):
        e0 = ge0 + tg * TILE_E
        bo = 2 * tg  # block offset within the group tiles

        # prescale: pres[:, 0, b, :] = x1*s2 ; pres[:, 1+m, b, :] = x1*v2m
        pres = pre.tile([128, 4, 2, 320], BF16)
        for b in range(2):
            for s in range(4):
                nc.vector.tensor_scalar_mul(
                    pres[:, s, b, :], x1s[:, bo + b, :], x2s[:, bo + b, s:s + 1])

        if VAR == 2:
            nc.scalar.copy(obsg[:, bo:bo + 2, :], pres[:, 0, :, :])
            continue

        # transposes to feature-major (PSUM, bf16)
        t1 = pt1.tile([128, 1024], BF16)
        t2 = pt2.tile([128, 1024], BF16)
        for b in range(2):
            o = 128 * b
            nc.tensor.transpose(t1[:, 0 + o:128 + o], pres[:, 0, b, 0:128], cId[:])
            nc.tensor.transpose(t1[:, 256 + o:384 + o], fws[:, bo + b, :], cId[:])
            nc.tensor.transpose(t1[:, 512 + o:640 + o], pres[:, 1, b, 0:128], cId[:])
            nc.tensor.transpose(t1[:, 768 + o:896 + o], pres[:, 2, b, 0:128], cId[:])
            nc.tensor.transpose(t2[:, 0 + o:128 + o], pres[:, 3, b, 0:128], cId[:])
            for m in range(3):
                # QTm (v1m*s2 planar) at rows 0:64, DTm (v1m*v2m) at rows 64:128
                oo = 256 * (m + 1) + o
                nc.tensor.transpose(
                    t2[0:64, oo:oo + 128], pres[:, 0, b, 128 + m:320:3], cId[:])
                nc.tensor.transpose(
                    t2[64:128, oo:oo + 128], pres[:, m + 1, b, 128 + m:320:3],
                    cId[:], tile_position=(0, 64))

        t1sb = evac.tile([128, 1024], BF16)
        t2sb = evac.tile([128, 1024], BF16)
        if SPLIT_EVAC:
            nc.vector.tensor_copy(t1sb[:, 0:512], t1[:, 0:512])
            nc.scalar.copy(t1sb[:, 512:1024], t1[:, 512:1024])
            nc.vector.tensor_copy(t2sb[:, 0:512], t2[:, 0:512])
            nc.scalar.copy(t2sb[:, 512:1024], t2[:, 512:1024])
        else:
            nc.vector.tensor_copy(t1sb[:], t1[:])
            nc.scalar.copy(t2sb[:], t2[:])

        if VAR == 3:
            nc.scalar.copy(obsg[:, bo:bo + 2, :],
                           t1sb[:, 0:640].rearrange("p (n d) -> p n d", n=2))
            continue

        PT = t1sb[:, 0:256]
        FT = t1sb[:, 256:512]
        RT = [t1sb[:, 512:768], t1sb[:, 768:1024], t2sb[:, 0:256]]
        QT = [t2sb[0:64, 256:512], t2sb[0:64, 512:768], t2sb[0:64, 768:1024]]
        DT = [t2sb[64:128, 256:512], t2sb[64:128, 512:768], t2sb[64:128, 768:1024]]

        mm1 = pm1.tile([128, 512], F32)   # out0 [128,0:256]; m0 [0:64,256:512]; h0 [64:128,256:512]
        mm2 = pm2.tile([128, 512], F32)   # m1 [0:64,0:256]; m2 [0:64,256:512]
        mm3 = pm3.tile([128, 512], F32)   # w0 [128,0:256]; h1 then w1 [0:64,256:512]

        # out0 = W1 over PT (K=128 rows 0:127) + W4 over DTm (K=64 rows 64:127)
        nc.tensor.matmul(mm1[:, 0:256], cW1[:], PT, start=True, stop=False)
        for m in range(3):
            nc.tensor.matmul(mm1[:, 0:256], c34[64:128, 0:128], DT[m],
                             start=False, stop=(m == 2), tile_position=(64, 0))

        # out1m = W2 over RTm + W3 over QTm, all at partitions 0:64
        o1 = [mm1[0:64, 256:512], mm2[0:64, 0:256], mm2[0:64, 256:512]]
        for m in range(3):
            nc.tensor.matmul(o1[m], cW2[:], RT[m], start=True, stop=False)
            nc.tensor.matmul(o1[m], c34[0:64, 0:64], QT[m], start=False, stop=True)

        w0s = fcs.tile([128, 256], F32)
        w1s = fcs.tile([64, 256], F32)
        if VAR != 6:
            # FC: h0 (at [64:128]) -> silu -> h1 (at [0:64]) -> silu -> w0/w1
            nc.tensor.matmul(mm1[64:128, 256:512], cF0[:], FT,
                             start=True, stop=True, tile_position=(0, 64))
            h0s = fcs.tile([128, 256], BF16)
            nc.scalar.activation(h0s[64:128, :], mm1[64:128, 256:512], ACT_FN)
            nc.tensor.matmul(mm3[0:64, 256:512], cF1[64:128, :], h0s[64:128, :],
                             start=True, stop=True, tile_position=(64, 0))
            h1s = fcs.tile([64, 256], BF16)
            nc.scalar.activation(h1s[:], mm3[0:64, 256:512], ACT_FN)
            nc.tensor.matmul(mm3[:, 0:256], cF2[:, 0:128], h1s[:],
                             start=True, stop=True)
            nc.tensor.matmul(mm3[0:64, 256:512], cF2[:, 128:192], h1s[:],
                             start=True, stop=True)
            nc.scalar.copy(w0s[:], mm3[:, 0:256])
            nc.scalar.copy(w1s[:], mm3[0:64, 256:512])
        else:
            nc.vector.memset(w0s[:], 1.0)
            nc.vector.memset(w1s[:], 1.0)

        # res = out * w  (feature-major, bf16 out); res1m all at partitions 0:64
        res0 = ress.tile([128, 256], BF16)
        nc.vector.tensor_mul(res0[:], mm1[:, 0:256], w0s[:])
        res1 = []
        for m in range(3):
            r1t = ress.tile([64, 256], BF16, tag=f"res1_{m}")
            res1.append(r1t)
        nc.vector.tensor_mul(res1[0][:], mm1[0:64, 256:512], w1s[:])
        nc.vector.tensor_mul(res1[1][:], mm2[0:64, 0:256], w1s[:])
        nc.vector.tensor_mul(res1[2][:], mm2[0:64, 256:512], w1s[:])

        if VAR == 4:
            nc.scalar.copy(obsg[:, bo, 0:256], res0[:])
            nc.scalar.copy(obsg[0:64, bo + 1, 0:256], res1[0][:])
            continue

        # transpose back to edge-major, m-planar 1o layout (host interleaves)
        ob = pob.tile([128, 640], BF16)
        for b in range(2):
            o = 320 * b
            ib = 128 * b
            nc.tensor.transpose(ob[:, o:o + 128], res0[:, ib:ib + 128], cId[:])
            for m in range(3):
                nc.tensor.transpose(ob[:, o + 128 + 64 * m:o + 192 + 64 * m],
                                    res1[m][:, ib:ib + 128], cId[0:64, 0:64])

        nc.scalar.copy(obsg[:, bo:bo + 2, :], ob[:].rearrange("p (n d) -> p n d", n=2))

      nc.gpsimd.dma_start(
          out=out[ge0:ge0 + GRP * TILE_E, :].rearrange("(n p) d -> p n d", p=128),
          in_=obsg[:])


def fold_weights(w1_1, w2_1, w1_2, w2_2, w1_3, w2_3, w1_4, w2_4,
                 fcw0, fcw1, fcw2):
    bf = ml_dtypes.bfloat16
    W1 = (w1_1 * w2_1 * C0).astype(bf)
    W2 = (w1_2 * w2_2 * (C1 * INV_SQRT3)).astype(bf)
    W3 = (w1_3 * w2_3 * (C1 * INV_SQRT3)).astype(bf)
    W4 = (w1_4 * w2_4 * (C0 * INV_SQRT3)).astype(bf)
    F0 = (fcw0 * (1.0 / np.sqrt(128.0))).astype(bf)
    F1 = (fcw1 * 0.125).astype(bf)
    F2 = (fcw2 * 0.125).astype(bf)
    return dict(wW1=W1, wW2=W2, wW3=W3, wW4=W4, wF0=F0, wF1=F1, wF2=F2,
                ident=np.eye(128, dtype=bf))


_nc = None


def prepare_in_maps(fea_in1, fea_in2, fea_weight,
                    w1_1, w2_1, w1_2, w2_2, w1_3, w2_3, w1_4, w2_4,
                    fcw0, fcw1, fcw2):
    wmap = fold_weights(np.asarray(w1_1, np.float32), np.asarray(w2_1, np.float32),
                        np.asarray(w1_2, np.float32), np.asarray(w2_2, np.float32),
                        np.asarray(w1_3, np.float32), np.asarray(w2_3, np.float32),
                        np.asarray(w1_4, np.float32), np.asarray(w2_4, np.float32),
                        np.asarray(fcw0, np.float32), np.asarray(fcw1, np.float32),
                        np.asarray(fcw2, np.float32))
    x1 = np.ascontiguousarray(np.asarray(fea_in1, np.float32))
    x2 = np.ascontiguousarray(np.asarray(fea_in2, np.float32))
    fwv = np.ascontiguousarray(np.asarray(fea_weight, np.float32))

    in_maps = []
    for c in range(N_CORES):
        sl = slice(c * E_CORE, (c + 1) * E_CORE)
        m = dict(x1=x1[sl], x2=x2[sl], fw=fwv[sl])
        m.update(wmap)
        in_maps.append(m)
    return in_maps


def run_spmd(in_maps, **kw):
    global _nc
    if _nc is None:
        _nc = build_nc()
    r = run_bass_kernel_spmd(_nc, in_maps, core_ids=list(range(N_CORES)), **kw)
    planar = np.concatenate([r.results[c]["out"] for c in range(N_CORES)], axis=0)
    return unplanarize(planar), r


def kernel(fea_in1, fea_in2, fea_weight, batch_edge,
           w1_1, w2_1, w1_2, w2_2, w1_3, w2_3, w1_4, w2_4,
           fcw0, fcw1, fcw2):
    in_maps = prepare_in_maps(fea_in1, fea_in2, fea_weight,
                              w1_1, w2_1, w1_2, w2_2, w1_3, w2_3, w1_4, w2_4,
                              fcw0, fcw1, fcw2)
    out, _ = run_spmd(in_maps)
    return out


def unplanarize(planar):
    # device emits 1o part m-planar ([.., m, w]); module layout interleaves
    # as 128+3w+m
    n = planar.shape[0]
    out = np.empty_like(planar)
    out[:, :128] = planar[:, :128]
    out[:, 128:] = planar[:, 128:].reshape(n, 3, 64).transpose(0, 2, 1).reshape(n, 192)
    return out



# revision 2
# speedup vs baseline: 1.0858x; 1.0858x over previous
# Bass/Tile kernel for nn_EquiConv (gnn_message_passing, memory-bound), v2.
#
# Math per edge e (weights pre-folded with e3nn norms + path scales):
#   A  = s1 @ W1            [128]   C  = s1 @ W2            [64]
#   dot= sum_m v1m * v2m    [64]    D4 = dot @ W4           [128]
#   rm = v1m * s2           [64x3]  Dm = rm @ W3            [64x3]
#   out0 = A*s2 + D4 ; out1m = C*v2m + Dm
#   h0 = silu(fw@F0); h1 = silu(h0@F1); w = h1@F2           [192]
#   res0 = out0 * w[:128]; res1m = out1m * w[128:192]
#
# Strategy: edge-data-parallel over 8 cores. Per core, groups of 1024 edges
# (8 blocks of 128). All matmuls are activation-stationary (lhsT = transposed
# activations, rhs = weights), so outputs land EDGE-major and per-edge scalars
# are per-partition ops. s1^T and fw^T come from HBM via XBAR DMA-transpose
# (bf16). dot/rm are prescaled on DVE edge-major, pair-pack-transposed on the
# PE into PSUM, evacuated to SBUF, then used as stationaries. PSUM is one
# [128, 4, 512] f32 quad-tile (4 banks) double-buffered; each block owns one
# bank with a choreographed region-reuse timeline.
import numpy as np
import ml_dtypes
from contextlib import ExitStack

import concourse.bass as bass
import concourse.tile as tile
from concourse import bacc, mybir
from concourse.bass_utils import run_bass_kernel_spmd

E_TOTAL = 262144
N_CORES = 8
E_CORE = E_TOTAL // N_CORES   # 32768
G = 1024                      # edges per group
NB = 8                        # 128-edge blocks per group
NQ = 2                        # quads per group (4 blocks each)
BF16 = mybir.dt.bfloat16
F32 = mybir.dt.float32
AF = mybir.ActivationFunctionType
OP = mybir.AluOpType

INV_SQRT3 = 1.0 / np.sqrt(3.0)
C0 = np.sqrt(1.0 / 192.0)
C1 = np.sqrt(3.0 / 192.0)

# region offsets within a block's 512-f32-word PSUM bank.
# timeline: dotT/rmT (bf16 words 384:896 of the bf16 view) -> evac1 ->
# D4 [192:320], Dm [320:512]; As2 consumes A -> h0 [0:64], h1 [64:128]
# (sequential); Cv consumes C -> h0T/h1T at bf16 [256:384] (sequential);
# out1-add consumes Dm -> w [320:512].
R_AC = 0      # f32 [0:192]   A|C
R_H0 = 0      # f32 [0:64]    h0 (after As2 read A)
R_H1 = 64     # f32 [64:128]  h1
R_HTB = 256   # bf16 [256:384] h0T then h1T (after Cv read C)
R_D4 = 192    # f32 [192:320] dotT(bf16 [384:512]) -> D4
R_DM = 320    # f32 [320:512] rmT(bf16 [512:896]) -> Dm -> w
R_W = 320


def build_nc(e_core=E_CORE, num_devices=N_CORES):
    nc = bacc.Bacc("TRN2", target_bir_lowering=False, debug=False,
                   num_devices=num_devices)
    xs = nc.dram_tensor("xs", [e_core, 128], BF16, kind="ExternalInput").ap()
    xvc = nc.dram_tensor("xvc", [e_core, 196], BF16, kind="ExternalInput").ap()
    fwd = nc.dram_tensor("fwd", [e_core, 128], BF16, kind="ExternalInput").ap()
    wAC = nc.dram_tensor("wAC", [128, 192], BF16, kind="ExternalInput").ap()
    wW4 = nc.dram_tensor("wW4", [128, 128], BF16, kind="ExternalInput").ap()
    wW3 = nc.dram_tensor("wW3", [128, 64], BF16, kind="ExternalInput").ap()
    wF0 = nc.dram_tensor("wF0", [128, 64], BF16, kind="ExternalInput").ap()
    wF1 = nc.dram_tensor("wF1", [128, 64], BF16, kind="ExternalInput").ap()
    wF2 = nc.dram_tensor("wF2", [128, 192], BF16, kind="ExternalInput").ap()
    ident = nc.dram_tensor("ident", [128, 128], BF16, kind="ExternalInput").ap()
    out = nc.dram_tensor("out", [e_core, 320], BF16, kind="ExternalOutput").ap()
    with tile.TileContext(nc) as tc, ExitStack() as ctx:
        _body(ctx, tc, xs, xvc, fwd,
              dict(wAC=wAC, wW4=wW4, wW3=wW3, wF0=wF0, wF1=wF1, wF2=wF2,
                   ident=ident),
              out, e_core)
    nc.compile()
    return nc


def _body(ctx, tc, xs, xvc, fwd, w_aps, out, e_core):
    nc = tc.nc
    ngroups = e_core // G

    const = ctx.enter_context(tc.tile_pool(name="const", bufs=1))
    cAC = const.tile([128, 192], BF16)
    cW4 = const.tile([128, 128], BF16)
    cW3 = const.tile([128, 64], BF16)
    cF0 = const.tile([128, 64], BF16)
    cF1 = const.tile([128, 64], BF16)
    cF2 = const.tile([128, 192], BF16)
    cId = const.tile([128, 128], BF16)
    for name, t in [("wAC", cAC), ("wW4", cW4), ("wW3", cW3), ("wF0", cF0),
                    ("wF1", cF1), ("wF2", cF2), ("ident", cId)]:
        nc.sync.dma_start(out=t[:], in_=w_aps[name])

    # SBUF pools
    p_xsT = ctx.enter_context(tc.tile_pool(name="p_xsT", bufs=2))
    p_fwT = ctx.enter_context(tc.tile_pool(name="p_fwT", bufs=2))
    p_xv = ctx.enter_context(tc.tile_pool(name="p_xv", bufs=2))
    p_pre = ctx.enter_context(tc.tile_pool(name="p_pre", bufs=2))
    p_ev = ctx.enter_context(tc.tile_pool(name="p_ev", bufs=3))
    p_hT = ctx.enter_context(tc.tile_pool(name="p_hT", bufs=3))
    p_mid = ctx.enter_context(tc.tile_pool(name="p_mid", bufs=3))
    p_res = ctx.enter_context(tc.tile_pool(name="p_res", bufs=2))
    # PSUM: one quad tile = 4 banks; bufs=2 -> 8 banks total
    p_ps = ctx.enter_context(tc.tile_pool(name="p_ps", bufs=2, space="PSUM"))

    # per-group state carried across the software pipeline
    st = {}

    def load_group(g):
        e0 = g * G
        xv = p_xv.tile([128, NB, 196], BF16, name=f"xv{g}", tag="xv")
        nc.gpsimd.dma_start(
            out=xv[:],
            in_=xvc[e0:e0 + G, :].rearrange("(n p) d -> p n d", p=128))
        xsT = p_xsT.tile([128, G], BF16, name=f"xsT{g}", tag="xsT")
        nc.sync.dma_start_transpose(xsT[:], xs[e0:e0 + G, :])
        fwT = p_fwT.tile([128, G], BF16, name=f"fwT{g}", tag="fwT")
        nc.sync.dma_start_transpose(fwT[:], fwd[e0:e0 + G, :])
        res = p_res.tile([128, NB, 320], BF16, name=f"res{g}", tag="res")
        st[g] = dict(xsT=xsT, fwT=fwT, xv=xv, res=res, ps={}, ev={}, h={})

    def sc(xv, blk_lo, blk_hi, j, width):
        # per-edge scalar column j broadcast to [128, nblk, width]
        return xv[:, blk_lo:blk_hi, 192 + j:193 + j].broadcast_to(
            (128, blk_hi - blk_lo, width))

    def prescale(g):
        # dot = sum_m v1m*v2m ; rm = v1m*s2   (group-grain, all SBUF bf16)
        xv = st[g]["xv"]
        dot = p_pre.tile([128, NB, 64], BF16, name=f"dot{g}", tag="dot")
        qa = p_pre.tile([128, NB, 64], BF16, name=f"qa{g}", tag="qa")
        qb = p_pre.tile([128, NB, 64], BF16, name=f"qb{g}", tag="qb")
        qc = p_pre.tile([128, NB, 64], BF16, name=f"qc{g}", tag="qc")
        rm = p_pre.tile([128, 3, NB, 64], BF16, name=f"rm{g}", tag="rm")
        nc.vector.tensor_tensor(qa[:], xv[:, :, 0:64], sc(xv, 0, NB, 1, 64), OP.mult)
        nc.vector.tensor_tensor(qb[:], xv[:, :, 64:128], sc(xv, 0, NB, 2, 64), OP.mult)
        nc.vector.tensor_tensor(qc[:], xv[:, :, 128:192], sc(xv, 0, NB, 3, 64), OP.mult)
        nc.vector.tensor_tensor(qa[:], qa[:], qb[:], OP.add)
        nc.vector.tensor_tensor(dot[:], qa[:], qc[:], OP.add)
        for m in range(3):
            nc.vector.tensor_tensor(rm[:, m, :, :], xv[:, :, 64 * m:64 * m + 64],
                                    sc(xv, 0, NB, 0, 64), OP.mult)
        st[g]["dot"] = dot
        st[g]["rm"] = rm

    def stage_T(g, q):
        # PE transposes of prescaled tensors into PSUM (pair-packed), plus
        # the A|C matmuls for this quad.
        dot, rm, xsT = st[g]["dot"], st[g]["rm"], st[g]["xsT"]
        ps = p_ps.tile([128, 4, 512], F32, name=f"ps{g}_{q}", tag="ps")
        st[g]["ps"][q] = ps
        for j in range(2):  # pair j within quad: blocks 4q+2j, 4q+2j+1
            b0 = 4 * q + 2 * j
            s_even = 2 * j
            nc.tensor.transpose(
                ps[:, s_even, 192:256].bitcast(BF16),
                dot[:, b0:b0 + 2, :], cId[:])
            for m in range(3):
                o = 256 + 64 * m
                nc.tensor.transpose(
                    ps[:, s_even, o:o + 64].bitcast(BF16),
                    rm[:, m, b0:b0 + 2, :], cId[:])
        for s in range(4):
            b = 4 * q + s
            nc.tensor.matmul(ps[:, s, 0:192], xsT[:, 128 * b:128 * b + 128],
                             cAC[:], start=True, stop=True)

    def stage_evac1(g, q):
        # PSUM -> SBUF evac of dotT/rmT (even banks, words 192:448)
        ps = st[g]["ps"][q]
        ev = p_ev.tile([128, 2, 512], BF16, name=f"ev{g}_{q}", tag="ev1")
        st[g]["ev"][q] = ev
        src = ps[:, 0:4:2, 192:448].bitcast(BF16)  # [128, 2, 512] dotT+rmT
        nc.vector.tensor_copy(ev[:, :, 0:256], src[:, :, 0:256])
        nc.scalar.copy(ev[:, :, 256:512], src[:, :, 256:512])

    def stage_mm(g, q):
        # D4 and Dm matmuls from evacuated stationaries
        ps = st[g]["ps"][q]
        ev = st[g]["ev"][q]
        for s in range(4):
            j, d = divmod(s, 2)
            tp = (64 * d, 0)
            lhs_dot = ev[64 * d:64 * d + 64, j, 0:128]
            nc.tensor.matmul(ps[:, s, R_D4:R_D4 + 128], lhs_dot,
                             cW4[64 * d:64 * d + 64, :], start=True, stop=True,
                             tile_position=tp)
            for m in range(3):
                lhs_rm = ev[64 * d:64 * d + 64, j, 128 + 128 * m:256 + 128 * m]
                nc.tensor.matmul(ps[:, s, R_DM + 64 * m:R_DM + 64 * m + 64],
                                 lhs_rm, cW3[64 * d:64 * d + 64, :],
                                 start=True, stop=True, tile_position=tp)

    def stage_combine1(g, q):
        # Cv (DVE quad), ATA0 (DVE per block) -> out0s
        xv = st[g]["xv"]
        ps = st[g]["ps"][q]
        cvs = p_mid.tile([128, 4, 3, 64], BF16, name=f"cvs{g}_{q}", tag="cvs")
        out0s = p_mid.tile([128, 4, 128], BF16, name=f"o0{g}_{q}", tag="o0")
        st[g][f"cvs{q}"] = cvs
        st[g][f"o0{q}"] = out0s
        as2 = p_mid.tile([128, 4, 128], BF16, name=f"as2_{g}_{q}", tag="as2")
        nc.vector.tensor_tensor(as2[:], ps[:, :, 0:128],
                                sc(xv, 4 * q, 4 * q + 4, 0, 128), OP.mult)
        for m in range(3):
            nc.vector.tensor_tensor(cvs[:, :, m, :], ps[:, :, 128:192],
                                    sc(xv, 4 * q, 4 * q + 4, 1 + m, 64), OP.mult)
        nc.vector.tensor_tensor(out0s[:], as2[:], ps[:, :, R_D4:R_D4 + 128],
                                OP.add)

    def stage_fc(g, q):
        # FC chain: F0 -> silu -> h0T -> evac -> F1 -> silu -> h1T -> evac -> F2
        ps = st[g]["ps"][q]
        psb = ps[:].bitcast(BF16)
        fwT = st[g]["fwT"]
        h0s = p_mid.tile([128, 4, 64], BF16, name=f"h0s{g}_{q}", tag="h0s")
        h1s = p_mid.tile([128, 4, 64], BF16, name=f"h1s{g}_{q}", tag="h1s")
        h0T = p_hT.tile([128, 2, 128], BF16, name=f"h0T{g}_{q}", tag="h0T")
        h1T = p_hT.tile([128, 2, 128], BF16, name=f"h1T{g}_{q}", tag="h1T")
        for s in range(4):
            b = 4 * q + s
            nc.tensor.matmul(ps[:, s, R_H0:R_H0 + 64], fwT[:, 128 * b:128 * b + 128],
                             cF0[:], start=True, stop=True)
        nc.scalar.activation(h0s[:], ps[:, :, R_H0:R_H0 + 64], AF.Silu)
        for j in range(2):
            nc.tensor.transpose(ps[:, 2 * j, 128:192].bitcast(BF16),
                                h0s[:, 2 * j:2 * j + 2, :], cId[:])
        nc.scalar.copy(h0T[:], ps[:, 0:4:2, 128:192].bitcast(BF16))
        for s in range(4):
            j, d = divmod(s, 2)
            nc.tensor.matmul(ps[:, s, R_H1:R_H1 + 64],
                             h0T[64 * d:64 * d + 64, j, :],
                             cF1[64 * d:64 * d + 64, :], start=True, stop=True,
                             tile_position=(64 * d, 0))
        nc.scalar.activation(h1s[:], ps[:, :, R_H1:R_H1 + 64], AF.Silu)
        for j in range(2):
            nc.tensor.transpose(ps[:, 2 * j, 128:192].bitcast(BF16),
                                h1s[:, 2 * j:2 * j + 2, :], cId[:])
        nc.scalar.copy(h1T[:], ps[:, 0:4:2, 128:192].bitcast(BF16))
        st[g][f"h1T{q}"] = h1T

    def stage_out1(g, q):
        # out1 = Cv + Dm (DVE quad) — must land before F2 overwrites Dm region
        ps = st[g]["ps"][q]
        cvs = st[g][f"cvs{q}"]
        out1s = p_mid.tile([128, 4, 3, 64], BF16, name=f"o1{g}_{q}", tag="o1")
        st[g][f"o1{q}"] = out1s
        dm = ps[:, :, R_DM:R_DM + 192].rearrange("p s (m w) -> p s m w", m=3)
        nc.vector.tensor_tensor(out1s[:], cvs[:], dm, OP.add)

    def stage_F2(g, q):
        ps = st[g]["ps"][q]
        h1T = st[g][f"h1T{q}"]
        for s in range(4):
            j, d = divmod(s, 2)
            nc.tensor.matmul(ps[:, s, R_W:R_W + 192],
                             h1T[64 * d:64 * d + 64, j, :],
                             cF2[64 * d:64 * d + 64, :], start=True, stop=True,
                             tile_position=(64 * d, 0))

    def stage_res(g, q):
        # w evac (Act, PSUM->SBUF), then res0 = out0*w0 (Pool, SBUF),
        # res1 = out1*w1 (DVE, SBUF)
        ps = st[g]["ps"][q]
        res = st[g]["res"]
        out0s = st[g][f"o0{q}"]
        out1s = st[g][f"o1{q}"]
        ws = p_mid.tile([128, 4, 192], BF16, name=f"ws{g}_{q}", tag="ws")
        nc.scalar.copy(ws[:], ps[:, :, R_W:R_W + 192])
        w1bc = ws[:, :, 128:192].rearrange("p s (m w) -> p s m w", m=1).broadcast_to(
            (128, 4, 3, 64))
        r1 = res[:, 4 * q:4 * q + 4, 128:320].rearrange(
            "p s (m w) -> p s m w", m=3)
        nc.vector.tensor_tensor(r1, out1s[:], w1bc, OP.mult)
        nc.gpsimd.tensor_tensor(res[:, 4 * q:4 * q + 4, 0:128], out0s[:],
                                ws[:, :, 0:128], OP.mult)

    def store_group(g):
        e0 = g * G
        nc.gpsimd.dma_start(
            out=out[e0:e0 + G, :].rearrange("(n p) d -> p n d", p=128),
            in_=st[g]["res"][:])

    import os
    VAR = int(os.environ.get("KN_VAR", "8"))

    # software pipeline over (group, quad) ticks with 1-quad skew between
    # the T/A|C stage and the dependent mm/FC stages.
    ticks = []
    for g in range(ngroups):
        for q in range(NQ):
            ticks.append((g, q))

    def emit_front(t):
        g, q = ticks[t]
        if q == 0:
            load_group(g)
            if VAR < 8:
                nc.vector.memset(st[g]["res"][:], 0.0)
        if q == 0 and VAR >= 1:
            prescale(g)
        if VAR >= 2:
            stage_T(g, q)
        if VAR >= 3:
            stage_evac1(g, q)
        if VAR < 3:
            st[g]["ev"][q] = None

    def emit_back(t):
        g, q = ticks[t]
        if VAR >= 4:
            stage_mm(g, q)
        if VAR >= 5:
            stage_combine1(g, q)
        if VAR >= 6:
            stage_fc(g, q)
        if VAR >= 7:
            stage_out1(g, q)
            stage_F2(g, q)
        if VAR >= 8:
            stage_res(g, q)
        if q == NQ - 1:
            if VAR >= 8:
                store_group(g)
            else:
                nc.gpsimd.dma_start(
                    out=out[g * G:(g + 1) * G, :].rearrange(
                        "(n p) d -> p n d", p=128),
                    in_=st[g]["res"][:])
            del st[g]["ps"], st[g]["ev"]

    SKEW = 1
    for t in range(len(ticks) + SKEW):
        if t < len(ticks):
            emit_front(t)
        if t >= SKEW:
            emit_back(t - SKEW)


def fold_weights(w1_1, w2_1, w1_2, w2_2, w1_3, w2_3, w1_4, w2_4,
                 fcw0, fcw1, fcw2):
    bf = ml_dtypes.bfloat16
    W1 = (w1_1 * w2_1 * C0)
    W2 = (w1_2 * w2_2 * (C1 * INV_SQRT3))
    W4 = (w1_4 * w2_4 * (C0 * INV_SQRT3))
    W3 = (w1_3 * w2_3 * (C1 * INV_SQRT3))
    dup = lambda w: np.vstack([w, w]).astype(bf)
    return dict(
        wAC=np.hstack([W1, W2]).astype(bf),
        wW4=dup(W4),
        wW3=dup(W3),
        wF0=(fcw0 * (1.0 / np.sqrt(128.0))).astype(bf),
        wF1=dup(fcw1 * 0.125),
        wF2=dup(fcw2 * 0.125),
        ident=np.eye(128, dtype=bf),
    )


_nc = None


def prepare_in_maps(fea_in1, fea_in2, fea_weight,
                    w1_1, w2_1, w1_2, w2_2, w1_3, w2_3, w1_4, w2_4,
                    fcw0, fcw1, fcw2):
    bf = ml_dtypes.bfloat16
    wmap = fold_weights(np.asarray(w1_1, np.float32), np.asarray(w2_1, np.float32),
                        np.asarray(w1_2, np.float32), np.asarray(w2_2, np.float32),
                        np.asarray(w1_3, np.float32), np.asarray(w2_3, np.float32),
                        np.asarray(w1_4, np.float32), np.asarray(w2_4, np.float32),
                        np.asarray(fcw0, np.float32), np.asarray(fcw1, np.float32),
                        np.asarray(fcw2, np.float32))
    x1 = np.asarray(fea_in1, np.float32)
    n = x1.shape[0]
    xs = np.ascontiguousarray(x1[:, :128]).astype(bf)
    # v planar [E, 3, 64] from interleaved [E, 64, 3], then append x2 scalars
    vpl = x1[:, 128:].reshape(n, 64, 3).transpose(0, 2, 1).reshape(n, 192)
    xvc = np.concatenate([vpl, np.asarray(fea_in2, np.float32)],
                         axis=1).astype(bf)
    xvc = np.ascontiguousarray(xvc)
    fwv = np.ascontiguousarray(np.asarray(fea_weight, np.float32)).astype(bf)

    in_maps = []
    for c in range(N_CORES):
        sl = slice(c * E_CORE, (c + 1) * E_CORE)
        m = dict(xs=xs[sl], xvc=xvc[sl], fwd=fwv[sl])
        m.update(wmap)
        in_maps.append(m)
    return in_maps


def run_spmd(in_maps, **kw):
    global _nc
    if _nc is None:
        _nc = build_nc()
    r = run_bass_kernel_spmd(_nc, in_maps, core_ids=list(range(N_CORES)), **kw)
    planar = np.concatenate(
        [np.asarray(r.results[c]["out"], dtype=np.float32) for c in range(N_CORES)],
        axis=0)
    return unplanarize(planar), r


def kernel(fea_in1, fea_in2, fea_weight, batch_edge,
           w1_1, w2_1, w1_2, w2_2, w1_3, w2_3, w1_4, w2_4,
           fcw0, fcw1, fcw2):
    in_maps = prepare_in_maps(fea_in1, fea_in2, fea_weight,
                              w1_1, w2_1, w1_2, w2_2, w1_3, w2_3, w1_4, w2_4,
                              fcw0, fcw1, fcw2)
    out, _ = run_spmd(in_maps)
    return out


def unplanarize(planar):
    # device emits 1o part m-planar ([.., m, w]); module layout interleaves
    # as 128+3w+m
    n = planar.shape[0]
    out = np.empty((n, 320), dtype=np.float32)
    out[:, :128] = planar[:, :128]
    out[:, 128:] = planar[:, 128:].reshape(n, 3, 64).transpose(0, 2, 1).reshape(n, 192)
    return out
